# revision 1
# baseline (speedup 1.0000x reference)
import sys

sys.path.insert(0, "/opt/trn_rl_repo")

import numpy as np

import concourse.bacc as bacc
import concourse.bass as bass
import concourse.mybir as mybir
import concourse.tile as tile
from concourse.bass_utils import run_bass_kernel_spmd

F32 = mybir.dt.float32

N, M, G, A, H = 20000, 48, 16, 64, 16
NCORES = 8
NL = N // NCORES  # 2500 atoms per core
CW = 160  # acat cols: [0:64]=a, [64:80]=gs, [80:160]=gv d-major 32-padded
NB2 = 256  # stage-2 group size (atoms)
NB1 = 8  # stage-1 subgroup size: 4 "even" (parts 0-47) + 4 "odd" (64-111)

_nc_cache = {}


def _dummy_mm(nc, out_ap, src_ap, tp):
    # K=1/M=1/N=1 matmul whose only job is to absorb one semaphore wait
    # (this walrus encodes at most one sync-wait per PE instruction).
    nc.tensor.matmul(
        out=out_ap, lhsT=src_ap, rhs=src_ap, start=True, stop=True, tile_position=tp
    )


def _build(nl=NL, nb2=NB2, sim=False):
    """Per-core Bass program.

    Stage 1 (per atom): psum1[32d+g, a] = sum_m gv[n,m,g,d]*a[n,m,a]
      (lhsT = acat gv-block [48m, 80], rhs = acat a-block [48m, 64]);
      atom pairs packed on PE row-group bases 0/64.
    Stage 1b: psum_s[a, g] = sum_m a[n,m,a]*gs[n,m,g].
    Stage 2 (per 256-atom group, channel a, axis d): K=16 matmul of
      agh[a] against vbig rows 32d..32d+16, 3x4 tile_position packing.
    Finish: ACT square, DVE d-sum, DVE 32x32 block-transpose -> n on
      partitions, 64B-chunk DMA out.
    """
    nc = bacc.Bacc("TRN2", target_bir_lowering=False)
    ac_d = nc.declare_dram_parameter("acat", [nl, M, CW], F32, isOutput=False)
    w_d = nc.declare_dram_parameter("aghw", [96, A * H], F32, isOutput=False)
    out_d = nc.declare_dram_parameter("out", [nl, A * G + A * H], F32, isOutput=True)

    Sq = mybir.ActivationFunctionType.Square

    with tile.TileContext(nc) as tc:
        with (
            tc.tile_pool(name="singles", bufs=1) as singles,
            tc.tile_pool(name="ain", bufs=4) as ain_pool,
            tc.tile_pool(name="vbig", bufs=1) as vbig_pool,
            tc.tile_pool(name="ssb", bufs=2) as ssb_pool,
            tc.tile_pool(name="sq", bufs=2) as sq_pool,
            tc.tile_pool(name="ov", bufs=2) as ov_pool,
            tc.tile_pool(name="ovt", bufs=2) as ovt_pool,
            tc.tile_pool(name="psum1", bufs=2, space="PSUM") as p1_pool,
            tc.tile_pool(name="psums", bufs=2, space="PSUM") as ps_pool,
            tc.tile_pool(name="psum2", bufs=1, space="PSUM") as p2_pool,
        ):
            aghw = singles.tile([96, A * H], F32)
            nc.sync.dma_start(out=aghw[:, :], in_=w_d[:, :])

            ngroups = (nl + nb2 - 1) // nb2
            for g2 in range(ngroups):
                n0 = g2 * nb2
                ncnt = min(nb2, nl - n0)
                vbig = vbig_pool.tile([80, nb2 * A], F32)

                # ---- stage 1 ----
                nflush = (ncnt + 31) // 32
                for fl in range(nflush):
                    fn0 = n0 + fl * 32
                    fcnt = min(32, n0 + ncnt - fn0)
                    psum_s = ps_pool.tile([128, 512], F32)
                    nsub = (fcnt + NB1 - 1) // NB1
                    first_sub = True
                    for sub in range(nsub):
                        sn0 = fn0 + sub * NB1
                        scnt = min(NB1, fn0 + fcnt - sn0)
                        acs = ain_pool.tile([128, NB1 * CW], F32)
                        nc.sync.dma_start(
                            out=acs[0:M, 0 : scnt * CW].rearrange(
                                "p (n c) -> p n c", c=CW
                            ),
                            in_=ac_d[sn0 : sn0 + scnt].rearrange("n m c -> m n c"),
                        )
                        psum1 = p1_pool.tile([128, NB1 * A], F32)
                        # wait absorbers (<=1 sync-wait per PE instruction):
                        # psum_s WAR (ACT), psum1 WAR (DVE), input DMA
                        if first_sub:
                            _dummy_mm(nc, psum_s[96:97, 0:1], aghw[0:1, 0:1], (0, 96))
                            first_sub = False
                        _dummy_mm(nc, psum1[96:97, 0:1], aghw[0:1, 0:1], (0, 96))
                        _dummy_mm(nc, psum1[96:97, 1:2], acs[0:1, 0:1], (0, 96))
                        for j in range(scnt):
                            c0 = j * CW
                            nc.tensor.matmul(
                                out=psum1[0:80, j * A : (j + 1) * A],
                                lhsT=acs[0:M, c0 + 80 : c0 + CW],
                                rhs=acs[0:M, c0 : c0 + A],
                                start=True,
                                stop=True,
                            )
                            je = sub * NB1 + j
                            nc.tensor.matmul(
                                out=psum_s[0:A, je * G : (je + 1) * G],
                                lhsT=acs[0:M, c0 : c0 + A],
                                rhs=acs[0:M, c0 + A : c0 + A + G],
                                start=True,
                                stop=True,
                            )
                        nc.vector.tensor_copy(
                            out=vbig[
                                :,
                                (fl * 32 + sub * NB1)
                                * A : (fl * 32 + sub * NB1 + scnt)
                                * A,
                            ],
                            in_=psum1[0:80, 0 : scnt * A],
                        )
                    s_sb = ssb_pool.tile([128, 512], F32)
                    nc.scalar.copy(
                        out=s_sb[0:A, 0 : fcnt * G], in_=psum_s[0:A, 0 : fcnt * G]
                    )
                    dst = out_d[fn0 : fn0 + fcnt, 0 : A * G].rearrange(
                        "n (a g) -> a n g", g=G
                    )
                    nc.sync.dma_start(
                        out=dst,
                        in_=s_sb[0:A, 0 : fcnt * G].rearrange(
                            "p (n g) -> p n g", g=G
                        ),
                    )

                # ---- stage 2: psum2 d-slices in separate banks ----
                rhs_full = vbig[:, :].rearrange("p (n a) -> p n a", a=A)
                for q in range(16):
                    psum2 = p2_pool.tile([128, 1536], F32)
                    _dummy_mm(nc, psum2[0:1, 300:301], aghw[0:1, 0:1], (0, 0))
                    for c in range(4):
                        a_ch = q * 4 + c
                        for d in range(3):
                            nc.tensor.matmul(
                                out=psum2[
                                    32 * c : 32 * c + H, d * 512 : d * 512 + ncnt
                                ],
                                lhsT=aghw[
                                    32 * d : 32 * d + G, a_ch * H : (a_ch + 1) * H
                                ],
                                rhs=rhs_full[
                                    32 * d : 32 * d + G, 0:ncnt, a_ch : a_ch + 1
                                ],
                                start=True,
                                stop=True,
                                tile_position=(32 * d, 32 * c),
                            )
                    if sim:
                        for r0 in (16, 48, 80):
                            for d in range(3):
                                nc.vector.memset(
                                    psum2[r0 : r0 + 16, d * 512 : d * 512 + 256], 0.0
                                )
                        if ncnt < 256:
                            for d in range(3):
                                nc.vector.memset(
                                    psum2[0:112, d * 512 + ncnt : d * 512 + 256], 0.0
                                )
                    sq = sq_pool.tile([128, 768], F32)
                    nc.scalar.activation(
                        out=sq[0:112, :],
                        in_=psum2[0:112, :].rearrange(
                            "p (d z) -> p d z", z=512
                        )[:, :, 0:256],
                        func=Sq,
                    )
                    ov = ov_pool.tile([128, 256], F32)
                    if sim:
                        nc.vector.memset(ov[96:128, 0:256], 0.0)
                    nc.vector.tensor_add(
                        ov[0:112, 0:ncnt],
                        sq[0:112, 0:ncnt],
                        sq[0:112, 256 : 256 + ncnt],
                    )
                    nc.vector.tensor_add(
                        ov[0:112, 0:ncnt],
                        ov[0:112, 0:ncnt],
                        sq[0:112, 512 : 512 + ncnt],
                    )
                    if ncnt < 256:
                        nc.vector.memset(ov[0:128, ncnt:256], 0.0)
                    ovt = ovt_pool.tile([128, 256], F32)
                    nc.vector.transpose(out=ovt[:, :], in_=ov[:, :])
                    # ovt[32c + n%32, 32*(n//32) + h] = out_v[n0+n, 4q+c, h]
                    nbfull = ncnt // 32
                    nrem = ncnt - nbfull * 32
                    for c in range(4):
                        a_ch = q * 4 + c
                        col0 = A * G + a_ch * H
                        if nbfull:
                            src = ovt[32 * c : 32 * c + 32, :].rearrange(
                                "p (b h2) -> p b h2", h2=32
                            )[:, 0:nbfull, 0:H]
                            dst = out_d[
                                n0 : n0 + nbfull * 32, col0 : col0 + H
                            ].rearrange("(b x) h -> x b h", x=32)
                            nc.sync.dma_start(out=dst, in_=src)
                        if nrem:
                            src = ovt[
                                32 * c : 32 * c + nrem,
                                nbfull * 32 : nbfull * 32 + H,
                            ]
                            dst = out_d[
                                n0 + nbfull * 32 : n0 + ncnt, col0 : col0 + H
                            ]
                            nc.sync.dma_start(out=dst, in_=src)
    nc.compile()
    return nc


def _get_nc():
    if "nc" not in _nc_cache:
        _nc_cache["nc"] = _build()
    return _nc_cache["nc"]


def _prep(a, gs, gv, agh):
    acat = np.zeros((N, M, CW), np.float32)
    acat[:, :, 0:A] = a
    acat[:, :, A : A + G] = gs
    for d in range(3):
        acat[:, :, 80 + 32 * d : 96 + 32 * d] = gv[:, :, :, d]
    aghw = np.zeros((96, A * H), np.float32)
    base = np.ascontiguousarray(np.asarray(agh, np.float32).transpose(1, 0, 2)).reshape(
        G, A * H
    )
    for r in range(3):
        aghw[32 * r : 32 * r + G] = base
    return acat, aghw


def kernel(a, gs, gv, agh):
    a = np.asarray(a, np.float32)
    gs = np.asarray(gs, np.float32)
    gv = np.asarray(gv, np.float32)
    acat, aghw = _prep(a, gs, gv, agh)
    nc = _get_nc()
    in_maps = [
        {"acat": acat[c * NL : (c + 1) * NL], "aghw": aghw} for c in range(NCORES)
    ]
    res = run_bass_kernel_spmd(nc, in_maps, list(range(NCORES))).results
    return np.concatenate([res[c]["out"] for c in range(NCORES)], axis=0)



# revision 7
# speedup vs baseline: 1.4461x; 1.4461x over previous
import sys

sys.path.insert(0, "/opt/trn_rl_repo")

import numpy as np

import concourse.bacc as bacc
import concourse.bass as bass
import concourse.mybir as mybir
import concourse.tile as tile
from concourse.bass_utils import run_bass_kernel_spmd

F32 = mybir.dt.float32
F16 = mybir.dt.float16

N, M, G, A, H = 20000, 48, 16, 64, 16
NCORES = 8
NL = N // NCORES  # 2500 atoms per core
CW = 160  # acat cols: [0:64]=a, [64:80]=gs, [80:160]=gv d-major 32-padded
NB2 = 256  # stage-2 group size (atoms)
NB1 = 8  # stage-1 subgroup size: 4 "even" (parts 0-47) + 4 "odd" (64-111)

_nc_cache = {}


def _dummy_mm(nc, out_ap, src_ap, tp):
    # K=1/M=1/N=1 matmul whose only job is to absorb one semaphore wait
    # (this walrus encodes at most one sync-wait per PE instruction).
    nc.tensor.matmul(
        out=out_ap, lhsT=src_ap, rhs=src_ap, start=True, stop=True, tile_position=tp
    )


def _build(nl=NL, nb2=NB2, sim=False):
    """Per-core Bass program.

    Stage 1 (per atom): psum1[32d+g, a] = sum_m gv[n,m,g,d]*a[n,m,a]
      (lhsT = acat gv-block [48m, 80], rhs = acat a-block [48m, 64]);
      atom pairs packed on PE row-group bases 0/64.
    Stage 1b: psum_s[a, g] = sum_m a[n,m,a]*gs[n,m,g].
    Stage 2 (per 256-atom group, channel a, axis d): K=16 matmul of
      agh[a] against vbig rows 32d..32d+16, 3x4 tile_position packing.
    Finish: ACT square, DVE d-sum, DVE 32x32 block-transpose -> n on
      partitions, 64B-chunk DMA out.
    """
    nc = bacc.Bacc("TRN2", target_bir_lowering=False)
    ac_d = nc.declare_dram_parameter("acat", [nl, M, CW], F16, isOutput=False)
    w_d = nc.declare_dram_parameter("aghw", [96, A * H], F16, isOutput=False)
    out_d = nc.declare_dram_parameter("out", [nl, A * G + A * H], F32, isOutput=True)

    Sq = mybir.ActivationFunctionType.Square

    with tile.TileContext(nc) as tc:
        with (
            tc.tile_pool(name="singles", bufs=1) as singles,
            tc.tile_pool(name="ain", bufs=4) as ain_pool,
            tc.tile_pool(name="vbig", bufs=1) as vbig_pool,
            tc.tile_pool(name="ssb", bufs=2) as ssb_pool,
            tc.tile_pool(name="sq", bufs=2) as sq_pool,
            tc.tile_pool(name="ov", bufs=2) as ov_pool,
            tc.tile_pool(name="ovt", bufs=2) as ovt_pool,
            tc.tile_pool(name="psum1", bufs=2, space="PSUM") as p1_pool,
            tc.tile_pool(name="psums", bufs=2, space="PSUM") as ps_pool,
            tc.tile_pool(name="psum2", bufs=1, space="PSUM") as p2_pool,
        ):
            aghw = singles.tile([96, A * H], F16)
            nc.sync.dma_start(out=aghw[:, :], in_=w_d[:, :])

            ngroups = (nl + nb2 - 1) // nb2
            for g2 in range(ngroups):
                n0 = g2 * nb2
                ncnt = min(nb2, nl - n0)
                vbig = vbig_pool.tile([80, nb2 * A], F16)

                # ---- stage 1 ----
                nflush = (ncnt + 31) // 32
                for fl in range(nflush):
                    fn0 = n0 + fl * 32
                    fcnt = min(32, n0 + ncnt - fn0)
                    psum_s = ps_pool.tile([128, 512], F32)
                    nsub = (fcnt + NB1 - 1) // NB1
                    first_sub = True
                    for sub in range(nsub):
                        sn0 = fn0 + sub * NB1
                        scnt = min(NB1, fn0 + fcnt - sn0)
                        acs = ain_pool.tile([128, NB1 * CW], F16)
                        nc.sync.dma_start(
                            out=acs[0:M, 0 : scnt * CW].rearrange(
                                "p (n c) -> p n c", c=CW
                            ),
                            in_=ac_d[sn0 : sn0 + scnt].rearrange("n m c -> m n c"),
                        )
                        psum1 = p1_pool.tile([128, NB1 * A], F32)
                        # wait absorbers (<=1 sync-wait per PE instruction):
                        # psum_s WAR (ACT), psum1 WAR (DVE), input DMA
                        if first_sub:
                            _dummy_mm(nc, psum_s[96:97, 0:1], aghw[0:1, 0:1], (0, 96))
                            first_sub = False
                        _dummy_mm(nc, psum1[96:97, 0:1], aghw[0:1, 0:1], (0, 96))
                        _dummy_mm(nc, psum1[96:97, 1:2], acs[0:1, 0:1], (0, 96))
                        for j in range(scnt):
                            c0 = j * CW
                            nc.tensor.matmul(
                                out=psum1[0:80, j * A : (j + 1) * A],
                                lhsT=acs[0:M, c0 + 80 : c0 + CW],
                                rhs=acs[0:M, c0 : c0 + A],
                                start=True,
                                stop=True,
                            )
                            je = sub * NB1 + j
                            nc.tensor.matmul(
                                out=psum_s[0:A, je * G : (je + 1) * G],
                                lhsT=acs[0:M, c0 : c0 + A],
                                rhs=acs[0:M, c0 + A : c0 + A + G],
                                start=True,
                                stop=True,
                            )
                        nc.vector.tensor_copy(
                            out=vbig[
                                :,
                                (fl * 32 + sub * NB1)
                                * A : (fl * 32 + sub * NB1 + scnt)
                                * A,
                            ],
                            in_=psum1[0:80, 0 : scnt * A],
                        )
                    s_sb = ssb_pool.tile([128, 512], F32)
                    nc.scalar.copy(
                        out=s_sb[0:A, 0 : fcnt * G], in_=psum_s[0:A, 0 : fcnt * G]
                    )
                    dst = out_d[fn0 : fn0 + fcnt, 0 : A * G].rearrange(
                        "n (a g) -> a n g", g=G
                    )
                    nc.sync.dma_start(
                        out=dst,
                        in_=s_sb[0:A, 0 : fcnt * G].rearrange(
                            "p (n g) -> p n g", g=G
                        ),
                    )

                # ---- stage 2: psum2 d-slices in separate banks ----
                rhs_full = vbig[:, :].rearrange("p (n a) -> p n a", a=A)
                for q in range(16):
                    psum2 = p2_pool.tile([128, 1536], F32)
                    _dummy_mm(nc, psum2[0:1, 300:301], aghw[0:1, 0:1], (0, 0))
                    for c in range(4):
                        a_ch = q * 4 + c
                        for d in range(3):
                            nc.tensor.matmul(
                                out=psum2[
                                    32 * c : 32 * c + H, d * 512 : d * 512 + ncnt
                                ],
                                lhsT=aghw[
                                    32 * d : 32 * d + G, a_ch * H : (a_ch + 1) * H
                                ],
                                rhs=rhs_full[
                                    32 * d : 32 * d + G, 0:ncnt, a_ch : a_ch + 1
                                ],
                                start=True,
                                stop=True,
                                tile_position=(32 * d, 32 * c),
                            )
                    if sim:
                        for r0 in (16, 48, 80):
                            for d in range(3):
                                nc.vector.memset(
                                    psum2[r0 : r0 + 16, d * 512 : d * 512 + 256], 0.0
                                )
                        if ncnt < 256:
                            for d in range(3):
                                nc.vector.memset(
                                    psum2[0:112, d * 512 + ncnt : d * 512 + 256], 0.0
                                )
                    sq = sq_pool.tile([128, 768], F32)
                    nc.scalar.activation(
                        out=sq[0:112, :],
                        in_=psum2[0:112, :].rearrange(
                            "p (d z) -> p d z", z=512
                        )[:, :, 0:256],
                        func=Sq,
                    )
                    ov = ov_pool.tile([128, 256], F32)
                    if sim:
                        nc.vector.memset(ov[96:128, 0:256], 0.0)
                    nc.vector.tensor_add(
                        ov[0:112, 0:ncnt],
                        sq[0:112, 0:ncnt],
                        sq[0:112, 256 : 256 + ncnt],
                    )
                    nc.vector.tensor_add(
                        ov[0:112, 0:ncnt],
                        ov[0:112, 0:ncnt],
                        sq[0:112, 512 : 512 + ncnt],
                    )
                    if ncnt < 256:
                        nc.vector.memset(ov[0:128, ncnt:256], 0.0)
                    ovt = ovt_pool.tile([128, 256], F32)
                    nc.vector.transpose(out=ovt[:, :], in_=ov[:, :])
                    # ovt[32c + n%32, 32*(n//32) + h] = out_v[n0+n, 4q+c, h]
                    nbfull = ncnt // 32
                    nrem = ncnt - nbfull * 32
                    for c in range(4):
                        a_ch = q * 4 + c
                        col0 = A * G + a_ch * H
                        if nbfull:
                            src = ovt[32 * c : 32 * c + 32, :].rearrange(
                                "p (b h2) -> p b h2", h2=32
                            )[:, 0:nbfull, 0:H]
                            dst = out_d[
                                n0 : n0 + nbfull * 32, col0 : col0 + H
                            ].rearrange("(b x) h -> x b h", x=32)
                            nc.sync.dma_start(out=dst, in_=src)
                        if nrem:
                            src = ovt[
                                32 * c : 32 * c + nrem,
                                nbfull * 32 : nbfull * 32 + H,
                            ]
                            dst = out_d[
                                n0 + nbfull * 32 : n0 + ncnt, col0 : col0 + H
                            ]
                            nc.sync.dma_start(out=dst, in_=src)
    nc.compile()
    return nc


def _get_nc():
    if "nc" not in _nc_cache:
        _nc_cache["nc"] = _build()
    return _nc_cache["nc"]


def _prep(a, gs, gv, agh):
    acat = np.zeros((N, M, CW), np.float16)
    acat[:, :, 0:A] = a
    acat[:, :, A : A + G] = gs
    for d in range(3):
        acat[:, :, 80 + 32 * d : 96 + 32 * d] = gv[:, :, :, d]
    aghw = np.zeros((96, A * H), np.float16)
    base = np.ascontiguousarray(np.asarray(agh, np.float32).transpose(1, 0, 2)).reshape(
        G, A * H
    )
    for r in range(3):
        aghw[32 * r : 32 * r + G] = base
    return acat, aghw


def kernel(a, gs, gv, agh):
    a = np.asarray(a, np.float32)
    gs = np.asarray(gs, np.float32)
    gv = np.asarray(gv, np.float32)
    acat, aghw = _prep(a, gs, gv, agh)
    nc = _get_nc()
    in_maps = [
        {"acat": acat[c * NL : (c + 1) * NL], "aghw": aghw} for c in range(NCORES)
    ]
    res = run_bass_kernel_spmd(nc, in_maps, list(range(NCORES))).results
    return np.concatenate([res[c]["out"] for c in range(NCORES)], axis=0)

